# revision 6
# baseline (speedup 1.0000x reference)
"""BG/NBD log-likelihood kernel for Trainium2 (8 NeuronCores, Bass/Tile).

Strategy
--------
x (repeat-transaction count) is a small non-negative integer, so every
lgamma term and the 2F1 series coefficients take only one value per class.
The host groups elements into rows of a fixed width F_B such that each row
is single-class, then lays rows out as [cores=8][groups][128 partitions].
Per-partition constant vectors carry the class-dependent coefficients, so
the device kernel is a short branch-free chain of big [128, F_B] ops:

    L1 = Ln(T + alpha)                       (ACT)
    u  = T - t_x ; L2 = Ln(u)                (DVE sub + ACT)
    L3 = Ln(t_x + alpha)                     (ACT)
    v  = L1 - L3          # = -log(1-z)      (DVE sub)
    G  ~= g4*v^4+g3*v^3+g2*v^2+g1*v+g0       degree-4 fit of log 2F1 in v,
         evaluated as beta*((v+h1)^2+h2)^2 + c1p*v (+const)  (2 ACT Squares)
    ll = beta*S2 + K0 + c1p*v + c*L2 - (c+r)*L1
         (1 dual tensor_scalar + 3 scalar_tensor_tensor, per-partition consts)

The degree-4 fit of G(v) = log 2F1(r+c, a; a+b+c; 1-e^-v) is accurate to
~5e-6 absolute because the v-substitution pushes the z=1 branch point to
infinity; measured end-to-end max relative error vs the f32 reference is
~1.4e-6. Class 0 rows use beta=c1p=c=0 so the same pipeline computes the
x==0 branch exactly. The fit runs on the host per call (it depends only on
the 4 scalar parameters and the per-class v-range; O(20) work).
"""
import sys

sys.path.insert(0, "/opt/trn_rl_repo")

import math

import numpy as np

import concourse.bass as bass
import concourse.bacc as bacc
import concourse.mybir as mybir
from concourse.tile import TileContext
from concourse import bass_utils

F32 = mybir.dt.float32
Alu = mybir.AluOpType
Act = mybir.ActivationFunctionType

N_CORES = 8
P = 128          # SBUF partitions
GROUPS = 4       # row-groups per core
R_TOT = N_CORES * GROUPS * P   # 4096 rows total


# --------------------------------------------------------------------------
# host-side math: per-class degree-4 fit of G(v) = log 2F1(...) in v
# --------------------------------------------------------------------------

def _hyp2f1_log_series(p, q, s, z, n_terms=500):
    """log of Gauss 2F1(p,q;s;z) via its power series, f64, vectorized in z."""
    term = np.ones_like(z)
    acc = np.ones_like(z)
    for k in range(n_terms):
        term = term * (p + k) * (q + k) / ((s + k) * (k + 1.0)) * z
        acc = acc + term
        if np.all(np.abs(term) < 1e-17 * np.abs(acc)):
            break
    return np.log(acc)


def _fit_class_params(c, vmin, vmax, r, a, b, log_alpha, deg=4, npts=512):
    """Return (h1, h2, beta, c1p, ccls, ncr, K0) for class c."""
    lg = math.lgamma
    if c == 0:
        K0 = r * log_alpha + math.log(b) - math.log(a + b)
        return (0.0, 0.0, 0.0, 0.0, 0.0, -r, K0)
    span = max(vmax - vmin, 1e-4)
    lo = max(vmin - 0.01 * span, 1e-7)
    hi = vmax + 0.01 * span
    v = np.linspace(lo, hi, npts)
    z = 1.0 - np.exp(-v)
    G = _hyp2f1_log_series(r + c, a, a + b + c, z)
    cheb = np.polynomial.chebyshev.Chebyshev.fit(v, G, deg)
    g = cheb.convert(kind=np.polynomial.Polynomial).coef
    g = np.concatenate([g, np.zeros(5 - len(g))]) if len(g) < 5 else g
    g0, g1, g2, g3, g4 = (float(t) for t in g[:5])
    if abs(g4) < 1e-18:
        g4 = 1e-18
    p_ = g3 / (2.0 * g4)
    q_ = (g2 / g4 - p_ * p_) / 2.0
    c1p = g1 - 2.0 * g4 * p_ * q_
    c0p = g0 - g4 * q_ * q_
    h1 = p_ / 2.0
    h2 = q_ - p_ * p_ / 4.0
    K_c = (lg(r + c) - lg(r) - lg(c + 1.0)
           + math.log(a) + lg(a + b) - lg(a)
           - lg(a + b + c) + lg(a + c)
           + r * log_alpha)
    return (h1, h2, g4, c1p, float(c), -(r + c), K_c + c0p)


# --------------------------------------------------------------------------
# device program (compiled once per (GROUPS, F_B); data-independent)
# --------------------------------------------------------------------------

_PROGRAM_CACHE = {}


def _build_program(groups, f_b):
    key = (groups, f_b)
    if key in _PROGRAM_CACHE:
        return _PROGRAM_CACHE[key]
    w = 2 * f_b + 8  # row layout: [T | t_x | consts]
    nc = bacc.Bacc("TRN2", target_bir_lowering=False, debug=False)
    Din = nc.dram_tensor("data_in", [groups, P, w], F32, kind="ExternalInput")
    Out = nc.dram_tensor("out", [groups, P, f_b], F32, kind="ExternalOutput")
    with TileContext(nc) as tc:
        with tc.tile_pool(name="io", bufs=2) as io, \
             tc.tile_pool(name="wk", bufs=2) as wk:
            for g in range(groups):
                IN = io.tile([P, w], F32, tag="in")
                nc.sync.dma_start(out=IN, in_=Din[g])
                tT = IN[:, 0:f_b]
                tX = IN[:, f_b:2 * f_b]
                cst = IN[:, 2 * f_b:w]
                L1 = wk.tile([P, f_b], F32, tag="L1")
                nc.scalar.activation(L1, tT, Act.Ln, bias=cst[:, 7:8], scale=1.0)
                L3 = wk.tile([P, f_b], F32, tag="L3")
                nc.scalar.activation(L3, tX, Act.Ln, bias=cst[:, 7:8], scale=1.0)
                # u = T - t_x (over the T slice), then L2 = Ln(u) in place
                nc.vector.tensor_tensor(out=tT, in0=tT, in1=tX, op=Alu.subtract)
                nc.scalar.activation(tT, tT, Act.Ln)
                # v = L1 - L3 (over L3)
                nc.vector.tensor_tensor(out=L3, in0=L1, in1=L3, op=Alu.subtract)
                # S2 = ((v + h1)^2 + h2)^2
                Sp = wk.tile([P, f_b], F32, tag="Sp")
                nc.scalar.activation(Sp, L3, Act.Square, bias=cst[:, 0:1], scale=1.0)
                nc.scalar.activation(Sp, Sp, Act.Square, bias=cst[:, 1:2], scale=1.0)
                # ll = beta*S2 + K0 + c1p*v + c*L2 + ncr*L1  (into the t_x slice)
                acc = wk.tile([P, f_b], F32, tag="acc")
                nc.vector.tensor_scalar(out=acc, in0=Sp, scalar1=cst[:, 2:3],
                                        scalar2=cst[:, 6:7], op0=Alu.mult, op1=Alu.add)
                nc.vector.scalar_tensor_tensor(out=acc, in0=L3, scalar=cst[:, 3:4],
                                               in1=acc, op0=Alu.mult, op1=Alu.add)
                nc.vector.scalar_tensor_tensor(out=acc, in0=tT, scalar=cst[:, 4:5],
                                               in1=acc, op0=Alu.mult, op1=Alu.add)
                nc.vector.scalar_tensor_tensor(out=tX, in0=L1, scalar=cst[:, 5:6],
                                               in1=acc, op0=Alu.mult, op1=Alu.add)
                nc.sync.dma_start(out=Out[g], in_=tX)
    nc.compile()
    _PROGRAM_CACHE[key] = nc
    return nc


# --------------------------------------------------------------------------
# kernel entry point
# --------------------------------------------------------------------------

def kernel(x, t_x, T, log_r, log_alpha, log_a, log_b, _trace=False):
    x = np.asarray(x)
    t_x = np.asarray(t_x, dtype=np.float32)
    T = np.asarray(T, dtype=np.float32)
    log_r = float(np.asarray(log_r))
    log_alpha = float(np.asarray(log_alpha))
    log_a = float(np.asarray(log_a))
    log_b = float(np.asarray(log_b))
    r = math.exp(log_r)
    alpha = math.exp(log_alpha)
    a = math.exp(log_a)
    b = math.exp(log_b)
    n = x.size

    # ---- group elements into single-class rows --------------------------
    order = np.argsort(x, kind="stable")
    xs = x[order]
    classes, starts, counts = np.unique(xs, return_index=True, return_counts=True)

    f_b = int(np.ceil(n / R_TOT / 8.0)) * 8
    while int(np.sum(np.ceil(counts / f_b))) > R_TOT:
        f_b += 8

    rows_per_class = np.ceil(counts / f_b).astype(np.int64)
    r_used = int(rows_per_class.sum())

    padded_idx = np.empty((R_TOT, f_b), dtype=np.int64)
    row_class = np.empty(R_TOT, dtype=np.int64)
    rr = 0
    for ci in range(len(classes)):
        idx = order[starts[ci]:starts[ci] + counts[ci]]
        nrows = int(rows_per_class[ci])
        cap = nrows * f_b
        pad = cap - idx.size
        if pad:
            idx = np.concatenate([idx, np.broadcast_to(idx[-1:], (pad,))])
        padded_idx[rr:rr + nrows] = idx.reshape(nrows, f_b)
        row_class[rr:rr + nrows] = classes[ci]
        rr += nrows
    if rr < R_TOT:  # fill spare rows with copies of the last real row
        padded_idx[rr:] = padded_idx[rr - 1]
        row_class[rr:] = row_class[rr - 1]

    # ---- per-class fit + per-row constants ------------------------------
    t64 = T.astype(np.float64)
    tx64 = t_x.astype(np.float64)
    v_all = np.log((alpha + t64) / (alpha + tx64))
    params = {}
    for ci, c in enumerate(classes):
        c = int(c)
        if c == 0:
            params[c] = _fit_class_params(0, 0.0, 1.0, r, a, b, log_alpha)
        else:
            sel = slice(starts[ci], starts[ci] + counts[ci])
            vc = v_all[order[sel]]
            params[c] = _fit_class_params(c, float(vc.min()), float(vc.max()),
                                          r, a, b, log_alpha)

    consts = np.empty((R_TOT, 8), dtype=np.float32)
    for c in np.unique(row_class):
        m = row_class == c
        consts[m, :7] = np.asarray(params[int(c)], dtype=np.float32)
    consts[:, 7] = np.float32(alpha)

    # ---- gather into device layout --------------------------------------
    w = 2 * f_b + 8  # per-row layout: [T | t_x | consts]
    data = np.empty((N_CORES, GROUPS, P, w), dtype=np.float32)
    data[..., 0:f_b] = T[padded_idx.ravel()].reshape(N_CORES, GROUPS, P, f_b)
    data[..., f_b:2 * f_b] = t_x[padded_idx.ravel()].reshape(N_CORES, GROUPS, P, f_b)
    data[..., 2 * f_b:w] = consts.reshape(N_CORES, GROUPS, P, 8)

    nc = _build_program(GROUPS, f_b)
    in_maps = [{"data_in": data[k]} for k in range(N_CORES)]
    run_kwargs = {}
    if _trace:
        run_kwargs = dict(trace=True, trace_cores=[0])
    res = bass_utils.run_bass_kernel_spmd(
        nc, in_maps, core_ids=list(range(N_CORES)), **run_kwargs)

    out_lay = np.stack([res.results[k]["out"] for k in range(N_CORES)])
    out_flat = out_lay.reshape(R_TOT * f_b)

    result = np.empty(n, dtype=np.float32)
    result[padded_idx.ravel()] = out_flat
    if _trace:
        kernel._last_trace = res
    return result


kernel._last_trace = None


# revision 7
# speedup vs baseline: 1.1919x; 1.1919x over previous
"""BG/NBD log-likelihood kernel for Trainium2 (8 NeuronCores, Bass/Tile).

Strategy
--------
x (repeat-transaction count) is a small non-negative integer, so every
lgamma term and the 2F1 series coefficients take only one value per class.
The host groups elements into rows of a fixed width F_B such that each row
is single-class, then lays rows out as [cores=8][groups][128 partitions].
Per-partition constant vectors carry the class-dependent coefficients, so
the device kernel is a short branch-free chain of big [128, F_B] ops:

    L1 = Ln(T + alpha)                       (ACT)
    u  = T - t_x ; L2 = Ln(u)                (DVE sub + ACT)
    L3 = Ln(t_x + alpha)                     (ACT)
    v  = L1 - L3          # = -log(1-z)      (DVE sub)
    G  ~= g4*v^4+g3*v^3+g2*v^2+g1*v+g0       degree-4 fit of log 2F1 in v,
         evaluated as beta*((v+h1)^2+h2)^2 + c1p*v (+const)  (2 ACT Squares)
    ll = beta*S2 + K0 + c1p*v + c*L2 - (c+r)*L1
         (1 dual tensor_scalar + 3 scalar_tensor_tensor, per-partition consts)

The degree-4 fit of G(v) = log 2F1(r+c, a; a+b+c; 1-e^-v) is accurate to
~5e-6 absolute because the v-substitution pushes the z=1 branch point to
infinity; measured end-to-end max relative error vs the f32 reference is
~1.4e-6. Class 0 rows use beta=c1p=c=0 so the same pipeline computes the
x==0 branch exactly. The fit runs on the host per call (it depends only on
the 4 scalar parameters and the per-class v-range; O(20) work).
"""
import sys

sys.path.insert(0, "/opt/trn_rl_repo")

import math

import numpy as np

import concourse.bass as bass
import concourse.bacc as bacc
import concourse.mybir as mybir
from concourse.tile import TileContext
from concourse import bass_utils

F32 = mybir.dt.float32
Alu = mybir.AluOpType
Act = mybir.ActivationFunctionType

N_CORES = 8
P = 128          # SBUF partitions
GROUPS = 4       # row-groups per core
R_TOT = N_CORES * GROUPS * P   # 4096 rows total


# --------------------------------------------------------------------------
# host-side math: per-class degree-4 fit of G(v) = log 2F1(...) in v
# --------------------------------------------------------------------------

def _hyp2f1_log_series(p, q, s, z, n_terms=500):
    """log of Gauss 2F1(p,q;s;z) via its power series, f64, vectorized in z."""
    term = np.ones_like(z)
    acc = np.ones_like(z)
    for k in range(n_terms):
        term = term * (p + k) * (q + k) / ((s + k) * (k + 1.0)) * z
        acc = acc + term
        if np.all(np.abs(term) < 1e-17 * np.abs(acc)):
            break
    return np.log(acc)


def _fit_class_params(c, vmin, vmax, r, a, b, log_alpha, deg=4, npts=512):
    """Return (h1, h2, beta, c1p, ccls, ncr, K0) for class c."""
    lg = math.lgamma
    if c == 0:
        K0 = r * log_alpha + math.log(b) - math.log(a + b)
        return (0.0, 0.0, 0.0, 0.0, 0.0, -r, K0)
    span = max(vmax - vmin, 1e-4)
    lo = max(vmin - 0.01 * span, 1e-7)
    hi = vmax + 0.01 * span
    v = np.linspace(lo, hi, npts)
    z = 1.0 - np.exp(-v)
    G = _hyp2f1_log_series(r + c, a, a + b + c, z)
    cheb = np.polynomial.chebyshev.Chebyshev.fit(v, G, deg)
    g = cheb.convert(kind=np.polynomial.Polynomial).coef
    g = np.concatenate([g, np.zeros(5 - len(g))]) if len(g) < 5 else g
    g0, g1, g2, g3, g4 = (float(t) for t in g[:5])
    if abs(g4) < 1e-18:
        g4 = 1e-18
    p_ = g3 / (2.0 * g4)
    q_ = (g2 / g4 - p_ * p_) / 2.0
    c1p = g1 - 2.0 * g4 * p_ * q_
    c0p = g0 - g4 * q_ * q_
    h1 = p_ / 2.0
    h2 = q_ - p_ * p_ / 4.0
    K_c = (lg(r + c) - lg(r) - lg(c + 1.0)
           + math.log(a) + lg(a + b) - lg(a)
           - lg(a + b + c) + lg(a + c)
           + r * log_alpha)
    return (h1, h2, g4, c1p, float(c), -(r + c), K_c + c0p)


# --------------------------------------------------------------------------
# device program (compiled once per (GROUPS, F_B); data-independent)
# --------------------------------------------------------------------------

_PROGRAM_CACHE = {}


def _build_program(groups, f_b):
    key = (groups, f_b)
    if key in _PROGRAM_CACHE:
        return _PROGRAM_CACHE[key]
    w = 2 * f_b + 8  # row layout: [T | t_x | consts]
    nc = bacc.Bacc("TRN2", target_bir_lowering=False, debug=False)
    Din = nc.dram_tensor("data_in", [groups, P, w], F32, kind="ExternalInput")
    Out = nc.dram_tensor("out", [groups, P, f_b], F32, kind="ExternalOutput")
    with TileContext(nc) as tc:
        with tc.tile_pool(name="io", bufs=3) as io, \
             tc.tile_pool(name="wk", bufs=3) as wk:
            for g in range(groups):
                IN = io.tile([P, w], F32, tag="in")
                nc.sync.dma_start(out=IN, in_=Din[g])
                tT = IN[:, 0:f_b]
                tX = IN[:, f_b:2 * f_b]
                cst = IN[:, 2 * f_b:w]
                L1 = wk.tile([P, f_b], F32, tag="L1")
                nc.scalar.activation(L1, tT, Act.Ln, bias=cst[:, 7:8], scale=1.0)
                L3 = wk.tile([P, f_b], F32, tag="L3")
                nc.scalar.activation(L3, tX, Act.Ln, bias=cst[:, 7:8], scale=1.0)
                # u = T - t_x (over the T slice), then L2 = Ln(u) in place
                nc.vector.tensor_tensor(out=tT, in0=tT, in1=tX, op=Alu.subtract)
                nc.scalar.activation(tT, tT, Act.Ln)
                # v = L1 - L3 (over L3)
                nc.vector.tensor_tensor(out=L3, in0=L1, in1=L3, op=Alu.subtract)
                # S2 = ((v + h1)^2 + h2)^2
                Sp = wk.tile([P, f_b], F32, tag="Sp")
                nc.scalar.activation(Sp, L3, Act.Square, bias=cst[:, 0:1], scale=1.0)
                nc.scalar.activation(Sp, Sp, Act.Square, bias=cst[:, 1:2], scale=1.0)
                # ll = beta*S2 + K0 + c1p*v + c*L2 + ncr*L1  (chained in place over Sp)
                nc.vector.tensor_scalar(out=Sp, in0=Sp, scalar1=cst[:, 2:3],
                                        scalar2=cst[:, 6:7], op0=Alu.mult, op1=Alu.add)
                nc.vector.scalar_tensor_tensor(out=Sp, in0=L3, scalar=cst[:, 3:4],
                                               in1=Sp, op0=Alu.mult, op1=Alu.add)
                nc.vector.scalar_tensor_tensor(out=Sp, in0=tT, scalar=cst[:, 4:5],
                                               in1=Sp, op0=Alu.mult, op1=Alu.add)
                nc.vector.scalar_tensor_tensor(out=tX, in0=L1, scalar=cst[:, 5:6],
                                               in1=Sp, op0=Alu.mult, op1=Alu.add)
                nc.sync.dma_start(out=Out[g], in_=tX)
    nc.compile()
    _PROGRAM_CACHE[key] = nc
    return nc


# --------------------------------------------------------------------------
# kernel entry point
# --------------------------------------------------------------------------

def kernel(x, t_x, T, log_r, log_alpha, log_a, log_b, _trace=False):
    x = np.asarray(x)
    t_x = np.asarray(t_x, dtype=np.float32)
    T = np.asarray(T, dtype=np.float32)
    log_r = float(np.asarray(log_r))
    log_alpha = float(np.asarray(log_alpha))
    log_a = float(np.asarray(log_a))
    log_b = float(np.asarray(log_b))
    r = math.exp(log_r)
    alpha = math.exp(log_alpha)
    a = math.exp(log_a)
    b = math.exp(log_b)
    n = x.size

    # ---- group elements into single-class rows --------------------------
    order = np.argsort(x, kind="stable")
    xs = x[order]
    classes, starts, counts = np.unique(xs, return_index=True, return_counts=True)

    f_b = int(np.ceil(n / R_TOT / 8.0)) * 8
    while int(np.sum(np.ceil(counts / f_b))) > R_TOT:
        f_b += 8

    rows_per_class = np.ceil(counts / f_b).astype(np.int64)
    r_used = int(rows_per_class.sum())

    padded_idx = np.empty((R_TOT, f_b), dtype=np.int64)
    row_class = np.empty(R_TOT, dtype=np.int64)
    rr = 0
    for ci in range(len(classes)):
        idx = order[starts[ci]:starts[ci] + counts[ci]]
        nrows = int(rows_per_class[ci])
        cap = nrows * f_b
        pad = cap - idx.size
        if pad:
            idx = np.concatenate([idx, np.broadcast_to(idx[-1:], (pad,))])
        padded_idx[rr:rr + nrows] = idx.reshape(nrows, f_b)
        row_class[rr:rr + nrows] = classes[ci]
        rr += nrows
    if rr < R_TOT:  # fill spare rows with copies of the last real row
        padded_idx[rr:] = padded_idx[rr - 1]
        row_class[rr:] = row_class[rr - 1]

    # ---- per-class fit + per-row constants ------------------------------
    t64 = T.astype(np.float64)
    tx64 = t_x.astype(np.float64)
    v_all = np.log((alpha + t64) / (alpha + tx64))
    params = {}
    for ci, c in enumerate(classes):
        c = int(c)
        if c == 0:
            params[c] = _fit_class_params(0, 0.0, 1.0, r, a, b, log_alpha)
        else:
            sel = slice(starts[ci], starts[ci] + counts[ci])
            vc = v_all[order[sel]]
            params[c] = _fit_class_params(c, float(vc.min()), float(vc.max()),
                                          r, a, b, log_alpha)

    consts = np.empty((R_TOT, 8), dtype=np.float32)
    for c in np.unique(row_class):
        m = row_class == c
        consts[m, :7] = np.asarray(params[int(c)], dtype=np.float32)
    consts[:, 7] = np.float32(alpha)

    # ---- gather into device layout --------------------------------------
    w = 2 * f_b + 8  # per-row layout: [T | t_x | consts]
    data = np.empty((N_CORES, GROUPS, P, w), dtype=np.float32)
    data[..., 0:f_b] = T[padded_idx.ravel()].reshape(N_CORES, GROUPS, P, f_b)
    data[..., f_b:2 * f_b] = t_x[padded_idx.ravel()].reshape(N_CORES, GROUPS, P, f_b)
    data[..., 2 * f_b:w] = consts.reshape(N_CORES, GROUPS, P, 8)

    nc = _build_program(GROUPS, f_b)
    in_maps = [{"data_in": data[k]} for k in range(N_CORES)]
    run_kwargs = {}
    if _trace:
        run_kwargs = dict(trace=True, trace_cores=[0])
    res = bass_utils.run_bass_kernel_spmd(
        nc, in_maps, core_ids=list(range(N_CORES)), **run_kwargs)

    out_lay = np.stack([res.results[k]["out"] for k in range(N_CORES)])
    out_flat = out_lay.reshape(R_TOT * f_b)

    result = np.empty(n, dtype=np.float32)
    result[padded_idx.ravel()] = out_flat
    if _trace:
        kernel._last_trace = res
    return result


kernel._last_trace = None
